# revision 1
# baseline (speedup 1.0000x reference)
"""Trainium2 Bass kernel for nn_CrossAttention (B=4, C=512, H=W=64, CQK=64).

Math (per batch b):
    Q = Wq @ rgb + bq                      [CQK, HW]
    K = Wk @ chm + bk                      [CQK, XY]
    V = Wv @ chm + bv                      [C, XY]
    S[hw, xy] = sum_o Q[o, hw] K[o, xy]    (xy = x*64 + y)
    P = softmax over y only (last 64-group of xy)
    att[c, hw] = sum_xy P[hw, xy] V[c, xy]
    out = rgb + gamma * att

Sharding: 8 cores = 4 batches x 2 halves of the hw (query) axis. Weights
replicated. Each core computes the full K/V for its batch and its 2048-row
slice of queries.

Device dataflow per core (all layouts channel/feature-major):
  - Qt[o, hw] (f32r), Kf[o, xy] (f32r) via 1x1-conv GEMMs; V^T[xy, c] (bf16).
  - S tiles [128 hw, xy] on PE (f32r), exp on ACT -> P (bf16, unnormalized),
    Z via DVE pairwise-tree sum over y, reciprocal, broadcast-multiply.
  - P^T via DMA xbar transpose (bf16), attend GEMM on PE (bf16),
    final add with rgb on DVE.
  - gamma and bv are folded on the host (bv contributes 64*gamma*bv[c] since
    softmax rows sum to 1 per (hw, x) and there are 64 x's).
DMA engine split: bulk loads on SWDGE (gpsimd), stores on the ACT HWDGE ring
(scalar), xbar transposes on the SP HWDGE ring (sync) to avoid single-FIFO
serialization.
"""

import numpy as np

import concourse.bass as bass
import concourse.mybir as mybir
import concourse.tile as tile
from concourse import bacc
from concourse.bass_utils import run_bass_kernel_spmd

P = 128
B, C, H, W = 4, 512, 64, 64
HW = H * W                # 4096
CQK = C // 8              # 64
N_CORES = 8
HWC = HW // 2             # hw rows per core (2048)

F32 = mybir.dt.float32
F32R = mybir.dt.float32r
BF16 = mybir.dt.bfloat16
ADD = mybir.AluOpType.add
MULT = mybir.AluOpType.mult
IDENT = mybir.ActivationFunctionType.Identity
EXP = mybir.ActivationFunctionType.Exp


def build_program(hwc=HWC, xy=HW, c=C, cqk=CQK, n_cores=N_CORES, repeat=1,
                  load_eng="gpsimd", store_eng="sync", ptb_bufs=2):
    """Build the per-core Bass program. Returns a compiled Bacc module."""
    ck = c // P               # channel chunks (4)
    nb = hwc // 512           # hw blocks (4)
    xt = xy // P              # xy tiles (32)
    xb = xy // 512            # xy 512-blocks (8)
    y = 64                    # softmax group size
    x_per_tile = xy // y      # x values (64 full size)

    nc = bacc.Bacc("TRN2", target_bir_lowering=False, debug=False,
                   num_devices=n_cores)
    ld = {"sync": nc.sync, "scalar": nc.scalar, "gpsimd": nc.gpsimd}[load_eng]
    st = {"sync": nc.sync, "scalar": nc.scalar, "gpsimd": nc.gpsimd}[store_eng]

    rgb = nc.dram_tensor("rgb", [c, hwc], F32, kind="ExternalInput")
    chm = nc.dram_tensor("chm", [c, xy], F32, kind="ExternalInput")
    wqT = nc.dram_tensor("wqT", [c, 2 * cqk], F32, kind="ExternalInput")
    wkT = nc.dram_tensor("wkT", [c, 2 * cqk], F32, kind="ExternalInput")
    wvT = nc.dram_tensor("wvT", [c, c], F32, kind="ExternalInput")
    bq = nc.dram_tensor("bq", [2 * cqk, 1], F32, kind="ExternalInput")
    bk = nc.dram_tensor("bk", [2 * cqk, 1], F32, kind="ExternalInput")
    out = nc.dram_tensor("out", [c, hwc], F32, kind="ExternalOutput")

    rgb_t = rgb.ap().rearrange("(k p) n -> p k n", p=P)
    chm_t = chm.ap().rearrange("(k p) n -> p k n", p=P)
    wq_t = wqT.ap().rearrange("(k p) m -> p k m", p=P)
    wk_t = wkT.ap().rearrange("(k p) m -> p k m", p=P)
    wv_t = wvT.ap().rearrange("(k p) m -> p k m", p=P)
    out_t = out.ap().rearrange("(k p) n -> p k n", p=P)

    with tile.TileContext(nc) as tc:
        with tc.tile_pool(name="persist", bufs=1) as pers:
            # --- weights / biases ---
            wq_r = pers.tile([P, ck, 2 * cqk], F32R)
            wk_r = pers.tile([P, ck, 2 * cqk], F32R)
            wv_b = pers.tile([P, ck, c], BF16)
            with tc.tile_pool(name="wload", bufs=1) as wload:
                wq_f = wload.tile([P, ck, 2 * cqk], F32)
                ld.dma_start(wq_f[:], wq_t)
                nc.vector.tensor_copy(wq_r[:], wq_f[:])
            bq_sb = pers.tile([2 * cqk, 1], F32)
            ld.dma_start(bq_sb[:], bq.ap())
            bk_sb = pers.tile([2 * cqk, 1], F32)
            ld.dma_start(bk_sb[:], bk.ap())

            qt_sb = pers.tile([2 * cqk, hwc], F32R)
            kf_sb = pers.tile([2 * cqk, xy], F32R)
            chmT_bf = pers.tile([P, xt, ck, P], BF16)

            for _rep in range(repeat):
                # deferred weight loads (not needed until Kf / att2)
                with tc.tile_pool(name="wload2", bufs=1) as wload2:
                    wk_f = wload2.tile([P, ck, 2 * cqk], F32)
                    ld.dma_start(wk_f[:], wk_t)
                    nc.vector.tensor_copy(wk_r[:], wk_f[:])
                    wv_f = wload2.tile([P, ck, c], F32)
                    ld.dma_start(wv_f[:], wv_t)
                    nc.vector.tensor_copy(wv_b[:], wv_f[:])

                # --- phase 1: Qt GEMM (rgb streamed) then Kf GEMM (chm
                # streamed); chm_bf shares the ptb tag: identical
                # 32KB/partition footprint, so phase 2's P^T buffers reuse its
                # slot once the chmT transposes are done.
                with tc.tile_pool(name="ptpool", bufs=ptb_bufs) as ptpool:
                    chm_bf = ptpool.tile([P, ck, xy], BF16, tag="ptb",
                                         name="chmbf")
                    half = xy // 2
                    with tc.tile_pool(name="qstream", bufs=2) as qstream, \
                         tc.tile_pool(name="psQ", bufs=1, space="PSUM") as psQ:
                        q_ps = [psQ.tile([2 * cqk, 512], F32, name=f"qps{i}")
                                for i in range(nb)]
                        for k in range(ck):
                            rf = qstream.tile([P, hwc], F32, tag="rf")
                            ld.dma_start(rf[:], rgb_t[:, k])
                            rr = qstream.tile([P, hwc], F32R, tag="rr")
                            nc.vector.tensor_copy(rr[:], rf[:])
                            for j in range(nb):
                                nc.tensor.matmul(
                                    q_ps[j][:], wq_r[:, k],
                                    rr[:, 512 * j:512 * (j + 1)],
                                    start=(k == 0), stop=(k == ck - 1))
                        for i in range(nb):
                            nc.scalar.activation(qt_sb[:, 512 * i:512 * (i + 1)],
                                                 q_ps[i][:], IDENT, bias=bq_sb[:])

                    with tc.tile_pool(name="stream", bufs=2) as stream, \
                         tc.tile_pool(name="psK", bufs=1, space="PSUM") as psK:
                        k_ps = [psK.tile([2 * cqk, 512], F32, name=f"kps{i}")
                                for i in range(xb)]
                        for k in range(ck):
                            for h in range(2):
                                cf = stream.tile([P, half], F32, tag="cf")
                                ld.dma_start(
                                    cf[:], chm_t[:, k, h * half:(h + 1) * half])
                                nc.scalar.copy(
                                    chm_bf[:, k, h * half:(h + 1) * half], cf[:])
                                cr = stream.tile([P, half], F32R, tag="cr")
                                nc.vector.tensor_copy(cr[:], cf[:])
                                for j in range(xb // 2):
                                    xblk = h * (xb // 2) + j
                                    nc.tensor.matmul(
                                        k_ps[xblk][:], wk_r[:, k],
                                        cr[:, 512 * j:512 * (j + 1)],
                                        start=(k == 0), stop=(k == ck - 1))
                        for i in range(xb):
                            nc.scalar.activation(kf_sb[:, 512 * i:512 * (i + 1)],
                                                 k_ps[i][:], IDENT, bias=bk_sb[:])

                    # chmT transposes: deferred so they fill DMA idle slots
                    # during the first softmax block (M1 needs them later).
                    for k in range(ck):
                        nc.sync.dma_start(chmT_bf[:, :, k, :], chm_bf[:, k],
                                          transpose=True)

                    # --- phase 2 (software-pipelined with V^T):
                    #     softmax(0) | V^T | softmax(b+1) interleaved with
                    #     attend(b) so PE fills gaps while ACT/DVE work ahead.
                    with tc.tile_pool(name="pmain", bufs=3) as pmain, \
                         tc.tile_pool(name="zpool", bufs=1) as zpool, \
                         tc.tile_pool(name="rgbf", bufs=1) as rgbf, \
                         tc.tile_pool(name="opool", bufs=2) as opool, \
                         tc.tile_pool(name="m1pool", bufs=2) as m1pool, \
                         tc.tile_pool(name="psS", bufs=2, space="PSUM") as psS, \
                         tc.tile_pool(name="psA", bufs=2, space="PSUM") as psA, \
                         nc.allow_low_precision(reason="softmax weights in bf16"):

                        def softmax_block(blk):
                            ptb = ptpool.tile([P, 4, xt, P], BF16, tag="ptb",
                                              name=f"ptb{blk}")
                            for ht in range(4):
                                htile = blk * 4 + ht
                                p_sb = pmain.tile([P, xy], BF16, tag="p")
                                for s in range(xy // 1024):
                                    s_ps = psS.tile([P, 1024], F32, tag="sps")
                                    # two K=64 matmuls packed into disjoint PE
                                    # row groups run concurrently in the array
                                    nc.tensor.matmul(
                                        s_ps[:, 0:512],
                                        qt_sb[0:cqk, P * htile:P * (htile + 1)],
                                        kf_sb[0:cqk, 1024 * s:1024 * s + 512],
                                        start=True, stop=True,
                                        tile_position=(0, 0))
                                    nc.tensor.matmul(
                                        s_ps[:, 512:1024],
                                        qt_sb[cqk:2 * cqk, P * htile:P * (htile + 1)],
                                        kf_sb[cqk:2 * cqk, 1024 * s + 512:1024 * (s + 1)],
                                        start=True, stop=True,
                                        tile_position=(cqk, 0))
                                    nc.scalar.activation(
                                        p_sb[:, 1024 * s:1024 * (s + 1)], s_ps[:], EXP)
                                # Z = sum over y (pairwise tree, bf16)
                                v3 = p_sb[:].rearrange("p (x y) -> p x y", y=y)
                                tcur = v3
                                w = y
                                while w > 1:
                                    w //= 2
                                    tnext = zpool.tile([P, x_per_tile, w], BF16,
                                                       tag=f"z{w}")
                                    nc.vector.tensor_tensor(
                                        tnext[:], tcur[:, :, 0:w], tcur[:, :, w:2 * w],
                                        ADD)
                                    tcur = tnext
                                rz = zpool.tile([P, x_per_tile, 1], BF16, tag="rz")
                                nc.vector.reciprocal(rz[:], tcur[:])
                                nc.vector.tensor_tensor(
                                    v3, v3, rz[:].to_broadcast([P, x_per_tile, y]),
                                    MULT)
                                nc.sync.dma_start(ptb[:, ht], p_sb[:], transpose=True)
                            return ptb

                        def attend_block(blk, ptb):
                            rg = rgbf.tile([P, ck, 512], F32, tag="rg")
                            ld.dma_start(rg[:],
                                         rgb_t[:, :, 512 * blk:512 * (blk + 1)])
                            # M1[cin, hw] = sum_xy chm[cin, xy] P^T[xy, hw]
                            m1_sb = m1pool.tile([P, ck, 512], BF16, tag="m1")
                            for ch in range(ck):
                                m_ps = psA.tile([P, 512], F32, tag="aps")
                                for m in range(xt):
                                    nc.tensor.matmul(
                                        m_ps[:], chmT_bf[:, m, ch, :],
                                        ptb[:, :, m, :],
                                        start=(m == 0), stop=(m == xt - 1))
                                nc.vector.tensor_copy(m1_sb[:, ch], m_ps[:])
                            # att[c, hw] = sum_cin (gamma Wv)[c, cin] M1[cin, hw]
                            o_sb = opool.tile([P, ck, 512], F32, tag="o")
                            for ct in range(ck):
                                a_ps = psA.tile([P, 512], F32, tag="aps")
                                for ch in range(ck):
                                    nc.tensor.matmul(
                                        a_ps[:], wv_b[:, ch, P * ct:P * (ct + 1)],
                                        m1_sb[:, ch],
                                        start=(ch == 0), stop=(ch == ck - 1))
                                nc.vector.tensor_tensor(o_sb[:, ct], a_ps[:],
                                                        rg[:, ct], ADD)
                            st.dma_start(out_t[:, :, 512 * blk:512 * (blk + 1)],
                                         o_sb[:])

                        ptbs = {0: softmax_block(0)}

                        for blk in range(1, nb):
                            ptbs[blk] = softmax_block(blk)
                            attend_block(blk - 1, ptbs.pop(blk - 1))
                        attend_block(nb - 1, ptbs.pop(nb - 1))

    nc.compile()
    return nc


_NC_CACHE = {}


def _get_nc():
    if "nc" not in _NC_CACHE:
        _NC_CACHE["nc"] = build_program()
    return _NC_CACHE["nc"]


def make_in_maps(rgb_features, chm_features, Wq, bq, Wk, bk, Wv, bv, gamma):
    rgb_features = np.asarray(rgb_features, dtype=np.float32)
    chm_features = np.asarray(chm_features, dtype=np.float32)
    Wq = np.asarray(Wq, dtype=np.float32)
    Wk = np.asarray(Wk, dtype=np.float32)
    Wv = np.asarray(Wv, dtype=np.float32)
    bq = np.asarray(bq, dtype=np.float32)
    bk = np.asarray(bk, dtype=np.float32)
    bv = np.asarray(bv, dtype=np.float32)
    g = float(np.asarray(gamma).reshape(-1)[0])

    wqT = np.ascontiguousarray(np.concatenate([Wq.T, Wq.T], axis=1))
    wkT = np.ascontiguousarray(np.concatenate([Wk.T, Wk.T], axis=1))
    wvT = np.ascontiguousarray((g * Wv).T)
    # softmax rows sum to 1 per (hw, x); summing over the 64 x's makes the
    # bias term contribute exactly 64*gamma*bv[c] to every output pixel.
    rgb_adj = rgb_features + (64.0 * g * bv)[None, :, None, None]
    bq2 = np.ascontiguousarray(np.concatenate([bq, bq]).reshape(2 * CQK, 1))
    bk2 = np.ascontiguousarray(np.concatenate([bk, bk]).reshape(2 * CQK, 1))

    in_maps = []
    for core in range(N_CORES):
        b, half = divmod(core, 2)
        rgb_c = np.ascontiguousarray(
            rgb_adj[b].reshape(C, HW)[:, half * HWC:(half + 1) * HWC])
        chm_c = np.ascontiguousarray(chm_features[b].reshape(C, HW))
        in_maps.append({
            "rgb": rgb_c, "chm": chm_c,
            "wqT": wqT, "wkT": wkT, "wvT": wvT,
            "bq": bq2, "bk": bk2,
        })
    return in_maps


def assemble(results):
    fused = np.empty((B, C, H, W), dtype=np.float32)
    fused2 = fused.reshape(B, C, HW)
    for core in range(N_CORES):
        b, half = divmod(core, 2)
        fused2[b, :, half * HWC:(half + 1) * HWC] = results[core]["out"]
    return fused


def kernel(rgb_features, chm_features, Wq, bq, Wk, bk, Wv, bv, gamma):
    nc = _get_nc()
    in_maps = make_in_maps(rgb_features, chm_features, Wq, bq, Wk, bk, Wv, bv,
                           gamma)
    res = run_bass_kernel_spmd(nc, in_maps, core_ids=list(range(N_CORES)))
    return assemble(res.results)



# revision 18
# speedup vs baseline: 1.1463x; 1.1463x over previous
"""Trainium2 Bass kernel for nn_CrossAttention (B=4, C=512, H=W=64, CQK=64).

Math (per batch b):
    Q = Wq @ rgb + bq                      [CQK, HW]
    K = Wk @ chm + bk                      [CQK, XY]
    S[hw, xy] = sum_o Q[o, hw] K[o, xy]
    P = softmax over y only (xy = x*64 + y)
    att[c, hw] = sum_xy P[hw, xy] (Wv @ chm + bv)[c, xy]
    out = rgb + gamma * att

Sharding: 8 cores = 4 batches x 2 halves of the hw (query) axis. Weights
replicated. Each core computes the full K for its batch and its 2048-row
slice of queries.

Device dataflow per core (key ideas vs the straightforward layout):
  - All big operands are pre-converted to bf16 on the host and chm's xy axis
    is pre-permuted to (y, x) order, so HBM traffic halves and no on-device
    dtype conversions or layout shuffles are needed.
  - Scores are computed TRANSPOSED: S^T[xy', hw] tiles with xy' on the
    partition axis. exp(S^T) then directly yields P~^T in the exact layout the
    attend GEMM wants as its moving operand - the 16 P^T DMA crossbar
    transposes of the untransposed scheme disappear entirely.
  - With xy' = y*64 + x, partition p of xy'-tile m holds y = 2m + p//64,
    x = p%64. The softmax y-sum becomes a free-dim pairwise tree over the 32
    m-tiles (DVE, 2x bf16 mode) plus one tiny PE matmul with a constant 0/1
    "comb" matrix that folds the two partition halves AND replicates the
    result across both halves: Z[p, hw] = z1[p%64, hw] + z1[p%64+64, hw].
  - The normalize multiply broadcasts 1/Z over the MIDDLE (m) axis, keeping
    the innermost axis packed so DVE runs it in 2x mode.
  - attend is reassociated: att = (gamma Wv) @ (chm @ P^T), so the big GEMM
    contracts chm[cin, xy] against P^T (8.6 GFLOP) and the 1x1 conv Wv is
    applied to the small result.
  - PSUM->SBUF copies and the final rgb adds run on the (otherwise idle)
    GPSIMD/Pool engine; exp on ACT; softmax reductions on DVE; PE does only
    matmuls. Score matmul pairs for block b+2 are woven one-pair-per-8
    attend matmuls of block b so ACT's exp throughput never stalls the PE
    queue and the PE stays at full p-state.
  - gamma and bv fold on the host (bv contributes 64*gamma*bv[c] since
    softmax rows sum to 1 per (hw, x) and there are 64 x's).
"""

import numpy as np

import concourse.bass as bass
import concourse.mybir as mybir
import concourse.tile as tile
from concourse import bacc
from concourse.bass_utils import run_bass_kernel_spmd

P = 128
B, C, H, W = 4, 512, 64, 64
HW = H * W                # 4096
CQK = C // 8              # 64
N_CORES = 8
HWC = HW // 2             # hw rows per core (2048)
XY = HW                   # key/value positions per batch (4096)

F32 = mybir.dt.float32
BF16 = mybir.dt.bfloat16
ADD = mybir.AluOpType.add
MULT = mybir.AluOpType.mult
IDENT = mybir.ActivationFunctionType.Identity
EXP = mybir.ActivationFunctionType.Exp


def build_program(hwc=HWC, xy=XY, c=C, cqk=CQK, n_cores=N_CORES):
    """Build the per-core Bass program. Returns a compiled Bacc module."""
    ck = c // P               # channel chunks (4)
    nb = hwc // 512           # hw blocks (4)
    xt = xy // P              # xy tiles (32)
    xb = xy // 512            # xy 512-blocks (8)

    nc = bacc.Bacc("TRN2", target_bir_lowering=False, debug=False,
                   num_devices=n_cores)
    ld = nc.gpsimd          # bulk loads + Pool-engine ALU offload
    st = nc.sync            # output stores
    wl = nc.scalar          # small weight loads (own HWDGE ring)

    rgb = nc.dram_tensor("rgb", [c, hwc], BF16, kind="ExternalInput")
    chm = nc.dram_tensor("chm", [c, xy], BF16, kind="ExternalInput")
    wq = nc.dram_tensor("wq", [c, 2 * cqk], BF16, kind="ExternalInput")
    wk = nc.dram_tensor("wk", [c, 2 * cqk], BF16, kind="ExternalInput")
    wv = nc.dram_tensor("wv", [c, c], BF16, kind="ExternalInput")
    bq = nc.dram_tensor("bq", [2 * cqk, 1], F32, kind="ExternalInput")
    bk = nc.dram_tensor("bk", [2 * cqk, 1], F32, kind="ExternalInput")
    comb = nc.dram_tensor("comb", [P, P], BF16, kind="ExternalInput")
    out = nc.dram_tensor("out", [c, hwc], F32, kind="ExternalOutput")

    rgb_t = rgb.ap().rearrange("(k p) n -> p k n", p=P)
    chm_t = chm.ap().rearrange("(k p) n -> p k n", p=P)
    wq_t = wq.ap().rearrange("(k p) m -> p k m", p=P)
    wk_t = wk.ap().rearrange("(k p) m -> p k m", p=P)
    wv_t = wv.ap().rearrange("(k p) m -> p k m", p=P)
    out_t = out.ap().rearrange("(k p) n -> p k n", p=P)

    with tile.TileContext(nc) as tc:
        with tc.tile_pool(name="pers", bufs=1) as pers, \
             tc.tile_pool(name="ptpool", bufs=3) as ptpool, \
             nc.allow_low_precision(reason="softmax weights in bf16"):
            # --- persistent tiles ---
            wq_sb = pers.tile([P, ck, 2 * cqk], BF16)
            wk_sb = pers.tile([P, ck, 2 * cqk], BF16)
            wv_sb = pers.tile([P, ck, c], BF16)
            comb_sb = pers.tile([P, P], BF16)
            bq_sb = pers.tile([2 * cqk, 1], F32)
            bk_sb = pers.tile([2 * cqk, 1], F32)
            qt_sb = pers.tile([2 * cqk, hwc], BF16)
            kf_sb = pers.tile([2 * cqk, xy], BF16)
            chmT = pers.tile([P, xt, ck, P], BF16)

            # small weight loads on the scalar HWDGE ring so the bulk gpsimd
            # ring starts streaming chm immediately
            wl.dma_start(wk_sb[:], wk_t)
            wl.dma_start(bk_sb[:], bk.ap())
            wl.dma_start(wq_sb[:], wq_t)
            wl.dma_start(bq_sb[:], bq.ap())
            wl.dma_start(comb_sb[:], comb.ap())

            with tc.tile_pool(name="chmp", bufs=1) as chmp, \
                 tc.tile_pool(name="rgbp", bufs=1) as rgbp:
                # --- Q GEMM first: qt[o, hw]; bias-add on DVE ---
                rgb_sb = [rgbp.tile([P, hwc], BF16, name=f"rgb{k}")
                          for k in range(ck)]
                for k in range(ck):
                    ld.dma_start(rgb_sb[k][:], rgb_t[:, k])
                with tc.tile_pool(name="psQ", bufs=1, space="PSUM") as psQ:
                    q_ps = [psQ.tile([2 * cqk, 512], F32, name=f"qps{i}")
                            for i in range(nb)]
                    for k in range(ck):
                        for j in range(nb):
                            nc.tensor.matmul(
                                q_ps[j][:], wq_sb[:, k],
                                rgb_sb[k][:, 512 * j:512 * (j + 1)],
                                start=(k == 0), stop=(k == ck - 1))
                    for j in range(nb):
                        nc.vector.tensor_scalar_add(
                            qt_sb[:, 512 * j:512 * (j + 1)], q_ps[j][:],
                            bq_sb[:])

                # --- K GEMM: kf[o, xy']; bias-adds on DVE so ACT does only
                # the softmax exps ---
                chm_sb = [chmp.tile([P, xy], BF16, name=f"chm{k}")
                          for k in range(ck)]
                for k in range(ck):
                    ld.dma_start(chm_sb[k][:], chm_t[:, k])
                with tc.tile_wait_until(0.022):
                    ld.dma_start(wv_sb[:], wv_t)
                with tc.tile_pool(name="psK", bufs=1, space="PSUM") as psK:
                    k_ps = [psK.tile([2 * cqk, 512], F32, name=f"kps{i}")
                            for i in range(xb)]
                    for k in range(ck):
                        for j in range(xb):
                            nc.tensor.matmul(
                                k_ps[j][:], wk_sb[:, k],
                                chm_sb[k][:, 512 * j:512 * (j + 1)],
                                start=(k == 0), stop=(k == ck - 1))
                    for j in range(xb):
                        nc.vector.tensor_scalar_add(
                            kf_sb[:, 512 * j:512 * (j + 1)], k_ps[j][:],
                            bk_sb[:])

                # chmT transposes, held back past the bulk loads so they do
                # not steal DMA-engine slots / semaphores from the critical
                # chm+rgb streams. Only needed by the first attend (~50us).
                # chmT[q, m, k, p] = chm[p, k, m*128+q]
                with tc.tile_wait_until(0.020):
                    for k in range(ck):
                        nc.scalar.dma_start(chmT[:, :, k, :], chm_sb[k][:],
                                            transpose=True)

            # --- phase 2 ---
            with tc.tile_pool(name="psS", bufs=2, space="PSUM") as psS, \
                 tc.tile_pool(name="psA", bufs=4, space="PSUM") as psA, \
                 tc.tile_pool(name="scr", bufs=1) as scrp, \
                 tc.tile_pool(name="rzp", bufs=2) as rzp, \
                 tc.tile_pool(name="m1p", bufs=2) as m1p, \
                 tc.tile_pool(name="op", bufs=2) as op, \
                 tc.tile_pool(name="rgp", bufs=2) as rgp:

                ptb = {}
                scr = {}
                rz = {}

                def score_pair(b, t):
                    """One pair of packed score matmuls + exp for block b."""
                    if t == 0:
                        ptb[b] = ptpool.tile([P, xt, 512], BF16, tag="ptb",
                                             name=f"ptb{b}")
                    flat = ptb[b][:].rearrange("p m n -> p (m n)")
                    s_ps = psS.tile([P, 1024], F32, tag="sps")
                    m0, m1_ = 2 * t, 2 * t + 1
                    nc.tensor.matmul(
                        s_ps[:, 0:512],
                        kf_sb[0:cqk, P * m0:P * (m0 + 1)],
                        qt_sb[0:cqk, 512 * b:512 * (b + 1)],
                        start=True, stop=True, tile_position=(0, 0))
                    nc.tensor.matmul(
                        s_ps[:, 512:1024],
                        kf_sb[cqk:2 * cqk, P * m1_:P * (m1_ + 1)],
                        qt_sb[cqk:2 * cqk, 512 * b:512 * (b + 1)],
                        start=True, stop=True, tile_position=(cqk, 0))
                    nc.scalar.activation(flat[:, 1024 * t:1024 * (t + 1)],
                                         s_ps[:], EXP)

                def tree(b):
                    """Pairwise y-sum over the 32 m-tiles (partial: halves)."""
                    s = scrp.tile([P, xt // 2, 512], BF16, tag="scr",
                                  name=f"scr{b}")
                    scr[b] = s
                    nc.vector.tensor_tensor(
                        s[:], ptb[b][:, 0:16], ptb[b][:, 16:32], ADD)
                    w = xt // 4
                    while w >= 1:
                        nc.vector.tensor_tensor(
                            s[:, 0:w], s[:, 0:w], s[:, w:2 * w], ADD)
                        w //= 2

                def zc_recip(b):
                    """Combine partition y-halves on PE, then reciprocal.
                    Z output borrows a psS slot (psA is full of M1 banks)."""
                    z_ps = psS.tile([P, 1024], F32, tag="sps", name=f"zps{b}")
                    nc.tensor.matmul(z_ps[:, 0:512], comb_sb[:], scr[b][:, 0],
                                     start=True, stop=True)
                    r = rzp.tile([P, 1, 512], BF16, tag="rz", name=f"rz{b}")
                    rz[b] = r
                    nc.vector.reciprocal(r[:], z_ps[:, 0:512])

                def norm_chunks(b):
                    """P~ *= 1/Z, split so the first attend m-tiles can start
                    before the whole block is normalized."""
                    for lo, hi in ((0, 8), (8, xt)):
                        nc.vector.tensor_tensor(
                            ptb[b][:, lo:hi], ptb[b][:, lo:hi],
                            rz[b][:].to_broadcast([P, hi - lo, 512]), MULT)

                def att2_store(b, m1_sb):
                    rg = rgp.tile([P, ck, 512], BF16, tag="rg", name=f"rg{b}")
                    ld.dma_start(rg[:], rgb_t[:, :, 512 * b:512 * (b + 1)])
                    o_sb = op.tile([P, ck, 512], F32, tag="o", name=f"o{b}")
                    for ct in range(ck):
                        a_ps = psA.tile([P, 512], F32, tag="aps",
                                        name=f"aps{b}_{ct}")
                        for ch in range(ck):
                            nc.tensor.matmul(
                                a_ps[:], wv_sb[:, ch, P * ct:P * (ct + 1)],
                                m1_sb[:, ch],
                                start=(ch == 0), stop=(ch == ck - 1))
                        nc.vector.tensor_tensor(o_sb[:, ct], a_ps[:],
                                                rg[:, ct], ADD)
                        if ct % 2 == 1:  # store in halves to start earlier
                            nc.sync.dma_start(
                                out_t[:, ct - 1:ct + 1,
                                      512 * b:512 * (b + 1)],
                                o_sb[:, ct - 1:ct + 1])

                # prologue: block 0 softmax + first half of block 1 scores
                for t in range(16):
                    score_pair(0, t)
                tree(0)
                for t in range(8):
                    score_pair(1, t)
                zc_recip(0)
                norm_chunks(0)

                # steady loop (lookahead-1): weave next block's score pairs
                # into this block's attend matmuls; M1 is m-major for the
                # first 8 m (racing the norm chunks) then ch-major so the
                # PSUM->SBUF copies stagger.
                weave = {b: [(b + 1, t) for t in range(16)] for b in range(nb)}
                weave[0] = [(1, t) for t in range(8, 16)]
                weave[nb - 1] = []
                for b in range(nb):
                    m1_sb = m1p.tile([P, ck, 512], BF16, tag="m1",
                                     name=f"m1_{b}")
                    pv = weave[b]
                    pi = 0
                    m_ps = [psA.tile([P, 512], F32, tag="aps",
                                     name=f"mps{b}_{ch}") for ch in range(ck)]
                    for m in range(8):
                        if pi < len(pv):
                            score_pair(*pv[pi])
                            pi += 1
                        for ch in range(ck):
                            nc.tensor.matmul(
                                m_ps[ch][:], chmT[:, m, ch, :], ptb[b][:, m],
                                start=(m == 0), stop=False)
                    for ch in range(ck):
                        for m in range(8, xt):
                            if ch < 2 and (m - 8) % 3 == 0 and pi < len(pv):
                                score_pair(*pv[pi])
                                pi += 1
                            nc.tensor.matmul(
                                m_ps[ch][:], chmT[:, m, ch, :], ptb[b][:, m],
                                start=False, stop=(m == xt - 1))
                        nc.scalar.copy(m1_sb[:, ch], m_ps[ch][:])
                    while pi < len(pv):
                        score_pair(*pv[pi])
                        pi += 1
                    if b + 1 < nb:
                        tree(b + 1)
                        zc_recip(b + 1)
                        norm_chunks(b + 1)
                    att2_store(b, m1_sb)

    nc.compile()
    return nc


_NC_CACHE = {}


def _get_nc():
    if "nc" not in _NC_CACHE:
        _NC_CACHE["nc"] = build_program()
    return _NC_CACHE["nc"]


def _bf16(a):
    import ml_dtypes
    return np.ascontiguousarray(a.astype(ml_dtypes.bfloat16))


def make_in_maps(rgb_features, chm_features, Wq, bq, Wk, bk, Wv, bv, gamma):
    rgb_features = np.asarray(rgb_features, dtype=np.float32)
    chm_features = np.asarray(chm_features, dtype=np.float32)
    Wq = np.asarray(Wq, dtype=np.float32)
    Wk = np.asarray(Wk, dtype=np.float32)
    Wv = np.asarray(Wv, dtype=np.float32)
    bq = np.asarray(bq, dtype=np.float32)
    bk = np.asarray(bk, dtype=np.float32)
    bv = np.asarray(bv, dtype=np.float32)
    g = float(np.asarray(gamma).reshape(-1)[0])

    wq2 = _bf16(np.concatenate([Wq.T, Wq.T], axis=1))
    wk2 = _bf16(np.concatenate([Wk.T, Wk.T], axis=1))
    wv2 = _bf16((g * Wv).T)
    # softmax rows sum to 1 per (hw, x); summing over the 64 x's makes the
    # bias term contribute exactly 64*gamma*bv[c] to every output pixel.
    rgb_adj = rgb_features + (64.0 * g * bv)[None, :, None, None]
    bq2 = np.ascontiguousarray(np.concatenate([bq, bq]).reshape(2 * CQK, 1))
    bk2 = np.ascontiguousarray(np.concatenate([bk, bk]).reshape(2 * CQK, 1))
    # comb[p, i] = (p % 64 == i % 64): folds the two partition y-halves of
    # the tree result and replicates across both halves in one matmul.
    comb = _bf16(np.tile(np.eye(CQK, dtype=np.float32), (2, 2)))

    in_maps = []
    for core in range(N_CORES):
        b, half = divmod(core, 2)
        rgb_c = _bf16(
            rgb_adj[b].reshape(C, HW)[:, half * HWC:(half + 1) * HWC])
        # chm with xy permuted to (y, x) order: col' = y*64 + x.
        chm_c = _bf16(chm_features[b].reshape(C, H, W)
                      .transpose(0, 2, 1).reshape(C, XY))
        in_maps.append({
            "rgb": rgb_c, "chm": chm_c,
            "wq": wq2, "wk": wk2, "wv": wv2,
            "bq": bq2, "bk": bk2, "comb": comb,
        })
    return in_maps


def assemble(results):
    fused = np.empty((B, C, H, W), dtype=np.float32)
    fused2 = fused.reshape(B, C, HW)
    for core in range(N_CORES):
        b, half = divmod(core, 2)
        fused2[b, :, half * HWC:(half + 1) * HWC] = results[core]["out"]
    return fused


def kernel(rgb_features, chm_features, Wq, bq, Wk, bk, Wv, bv, gamma):
    nc = _get_nc()
    in_maps = make_in_maps(rgb_features, chm_features, Wq, bq, Wk, bk, Wv, bv,
                           gamma)
    res = run_bass_kernel_spmd(nc, in_maps, core_ids=list(range(N_CORES)))
    return assemble(res.results)


# revision 27
# speedup vs baseline: 1.2434x; 1.0848x over previous
"""Trainium2 Bass kernel for nn_CrossAttention (B=4, C=512, H=W=64, CQK=64).

Math (per batch b):
    Q = Wq @ rgb + bq                      [CQK, HW]
    K = Wk @ chm + bk                      [CQK, XY]
    S[hw, xy] = sum_o Q[o, hw] K[o, xy]
    P = softmax over y only (xy = x*64 + y)
    att[c, hw] = sum_xy P[hw, xy] (Wv @ chm + bv)[c, xy]
    out = rgb + gamma * att

Sharding: 8 cores = 4 batches x 2 halves of the hw (query) axis. Weights
replicated. Each core computes the full K for its batch and its 2048-row
slice of queries.

Device dataflow per core (key ideas vs the straightforward layout):
  - All big operands are pre-converted to bf16 on the host and chm's xy axis
    is pre-permuted to (y, x) order, so HBM traffic halves and no on-device
    dtype conversions or layout shuffles are needed.
  - Scores are computed TRANSPOSED: S^T[xy', hw] tiles with xy' on the
    partition axis. exp(S^T) then directly yields P~^T in the exact layout the
    attend GEMM wants as its moving operand - the 16 P^T DMA crossbar
    transposes of the untransposed scheme disappear entirely.
  - With xy' = y*64 + x, partition p of xy'-tile m holds y = 2m + p//64,
    x = p%64. The softmax y-sum becomes a free-dim pairwise tree over the 32
    m-tiles (DVE, 2x bf16 mode) plus one tiny PE matmul with a constant 0/1
    "comb" matrix that folds the two partition halves AND replicates the
    result across both halves: Z[p, hw] = z1[p%64, hw] + z1[p%64+64, hw].
  - The normalize multiply broadcasts 1/Z over the MIDDLE (m) axis, keeping
    the innermost axis packed so DVE runs it in 2x mode.
  - attend is reassociated: att = (gamma Wv) @ (chm @ P^T), so the big GEMM
    contracts chm[cin, xy] against P^T (8.6 GFLOP) and the 1x1 conv Wv is
    applied to the small result.
  - PSUM->SBUF copies and the final rgb adds run on the (otherwise idle)
    GPSIMD/Pool engine; exp on ACT; softmax reductions on DVE; PE does only
    matmuls. Score matmul pairs for block b+2 are woven one-pair-per-8
    attend matmuls of block b so ACT's exp throughput never stalls the PE
    queue and the PE stays at full p-state.
  - gamma and bv fold on the host (bv contributes 64*gamma*bv[c] since
    softmax rows sum to 1 per (hw, x) and there are 64 x's).
"""

import numpy as np

import concourse.bass as bass
import concourse.mybir as mybir
import concourse.tile as tile
from concourse import bacc
from concourse.bass_utils import run_bass_kernel_spmd

P = 128
B, C, H, W = 4, 512, 64, 64
HW = H * W                # 4096
CQK = C // 8              # 64
N_CORES = 8
HWC = HW // 2             # hw rows per core (2048)
XY = HW                   # key/value positions per batch (4096)

F32 = mybir.dt.float32
BF16 = mybir.dt.bfloat16
ADD = mybir.AluOpType.add
MULT = mybir.AluOpType.mult
IDENT = mybir.ActivationFunctionType.Identity
EXP = mybir.ActivationFunctionType.Exp


def build_program(hwc=HWC, xy=XY, c=C, cqk=CQK, n_cores=N_CORES):
    """Build the per-core Bass program. Returns a compiled Bacc module."""
    ck = c // P               # channel chunks (4)
    nb = hwc // 512           # hw blocks (4)
    xt = xy // P              # xy tiles (32)
    xb = xy // 512            # xy 512-blocks (8)

    nc = bacc.Bacc("TRN2", target_bir_lowering=False, debug=False,
                   num_devices=n_cores)
    ld = nc.gpsimd          # bulk loads + Pool-engine ALU offload
    st = nc.sync            # output stores
    wl = nc.scalar          # small weight loads (own HWDGE ring)

    rgb = nc.dram_tensor("rgb", [c, hwc], BF16, kind="ExternalInput")
    chm = nc.dram_tensor("chm", [c, xy], BF16, kind="ExternalInput")
    wq = nc.dram_tensor("wq", [c, 2 * cqk], BF16, kind="ExternalInput")
    wk = nc.dram_tensor("wk", [c, 2 * cqk], BF16, kind="ExternalInput")
    wv = nc.dram_tensor("wv", [c, c], BF16, kind="ExternalInput")
    bq = nc.dram_tensor("bq", [2 * cqk, 1], F32, kind="ExternalInput")
    bk = nc.dram_tensor("bk", [2 * cqk, 1], F32, kind="ExternalInput")
    comb = nc.dram_tensor("comb", [P, P], BF16, kind="ExternalInput")
    out = nc.dram_tensor("out", [c, hwc], F32, kind="ExternalOutput")

    rgb_t = rgb.ap().rearrange("(k p) n -> p k n", p=P)
    chm_t = chm.ap().rearrange("(k p) n -> p k n", p=P)
    wq_t = wq.ap().rearrange("(k p) m -> p k m", p=P)
    wk_t = wk.ap().rearrange("(k p) m -> p k m", p=P)
    wv_t = wv.ap().rearrange("(k p) m -> p k m", p=P)
    out_t = out.ap().rearrange("(k p) n -> p k n", p=P)

    with tile.TileContext(nc) as tc:
        with tc.tile_pool(name="pers", bufs=1) as pers, \
             tc.tile_pool(name="ptpool", bufs=3) as ptpool, \
             nc.allow_low_precision(reason="softmax weights in bf16"):
            # --- persistent tiles ---
            wq_sb = pers.tile([P, ck, 2 * cqk], BF16)
            wk_sb = pers.tile([P, ck, 2 * cqk], BF16)
            wv_sb = pers.tile([P, ck, c], BF16)
            comb_sb = pers.tile([P, P], BF16)
            bq_sb = pers.tile([2 * cqk, 1], F32)
            bk_sb = pers.tile([2 * cqk, 1], F32)
            qt_sb = pers.tile([2 * cqk, hwc], BF16)
            kf_sb = pers.tile([2 * cqk, xy], BF16)
            chmT = pers.tile([P, xt, ck, P], BF16)

            # small weight loads on the scalar HWDGE ring so the bulk gpsimd
            # ring starts streaming rgb/chm immediately
            wl.dma_start(wq_sb[:], wq_t)
            wl.dma_start(bq_sb[:], bq.ap())
            wl.dma_start(wk_sb[:], wk_t)
            wl.dma_start(bk_sb[:], bk.ap())
            wl.dma_start(comb_sb[:], comb.ap())

            with tc.tile_pool(name="chmp", bufs=1) as chmp, \
                 tc.tile_pool(name="rgbp", bufs=1) as rgbp:
                # --- Q GEMM first: qt[o, hw]; bias-add on DVE ---
                rgb_sb = [rgbp.tile([P, hwc], BF16, name=f"rgb{k}")
                          for k in range(ck)]
                for k in range(ck):
                    ld.dma_start(rgb_sb[k][:], rgb_t[:, k])
                with tc.tile_pool(name="psQ", bufs=1, space="PSUM") as psQ:
                    q_ps = [psQ.tile([2 * cqk, 512], F32, name=f"qps{i}")
                            for i in range(nb)]
                    for k in range(ck):
                        for j in range(nb):
                            nc.tensor.matmul(
                                q_ps[j][:], wq_sb[:, k],
                                rgb_sb[k][:, 512 * j:512 * (j + 1)],
                                start=(k == 0), stop=(k == ck - 1))
                    for j in range(nb):
                        nc.vector.tensor_scalar_add(
                            qt_sb[:, 512 * j:512 * (j + 1)], q_ps[j][:],
                            bq_sb[:])

                # --- K GEMM: kf[o, xy']; bias-adds split ACT/DVE so the
                # first score pairs start as early as possible ---
                chm_sb = [chmp.tile([P, xy], BF16, name=f"chm{k}")
                          for k in range(ck)]
                for k in range(ck):
                    ld.dma_start(chm_sb[k][:], chm_t[:, k])
                with tc.tile_wait_until(0.022):
                    ld.dma_start(wv_sb[:], wv_t)
                with tc.tile_pool(name="psK", bufs=1, space="PSUM") as psK:
                    k_ps = [psK.tile([2 * cqk, 512], F32, name=f"kps{i}")
                            for i in range(xb)]
                    for k in range(ck):
                        for j in range(xb):
                            nc.tensor.matmul(
                                k_ps[j][:], wk_sb[:, k],
                                chm_sb[k][:, 512 * j:512 * (j + 1)],
                                start=(k == 0), stop=(k == ck - 1))
                    for j in range(xb):
                        if j < 4:
                            nc.scalar.activation(
                                kf_sb[:, 512 * j:512 * (j + 1)], k_ps[j][:],
                                IDENT, bias=bk_sb[:])
                        else:
                            nc.vector.tensor_scalar_add(
                                kf_sb[:, 512 * j:512 * (j + 1)], k_ps[j][:],
                                bk_sb[:])

                # chmT transposes on the sync ring, held back past the bulk
                # loads (tile_wait_until) so they do not steal DMA-engine
                # slots / semaphores from the critical chm+rgb streams, and
                # off the ACT ring so they cannot delay the softmax exps.
                # Only needed by the first attend (~45us).
                # chmT[q, m, k, p] = chm[p, k, m*128+q]
                with tc.tile_wait_until(0.020):
                    for k in range(ck):
                        nc.sync.dma_start(chmT[:, :, k, :], chm_sb[k][:],
                                          transpose=True)

            # --- phase 2 ---
            with tc.tile_pool(name="psS", bufs=2, space="PSUM") as psS, \
                 tc.tile_pool(name="psA", bufs=4, space="PSUM") as psA, \
                 tc.tile_pool(name="scr", bufs=1) as scrp, \
                 tc.tile_pool(name="rzp", bufs=2) as rzp, \
                 tc.tile_pool(name="m1p", bufs=2) as m1p, \
                 tc.tile_pool(name="op", bufs=2) as op, \
                 tc.tile_pool(name="rgp", bufs=2) as rgp:

                ptb = {}
                scr = {}
                rz = {}

                def score_pair(b, t):
                    """One pair of packed score matmuls + exp for block b."""
                    if t == 0:
                        ptb[b] = ptpool.tile([P, xt, 512], BF16, tag="ptb",
                                             name=f"ptb{b}")
                    flat = ptb[b][:].rearrange("p m n -> p (m n)")
                    s_ps = psS.tile([P, 1024], F32, tag="sps")
                    m0, m1_ = 2 * t, 2 * t + 1
                    nc.tensor.matmul(
                        s_ps[:, 0:512],
                        kf_sb[0:cqk, P * m0:P * (m0 + 1)],
                        qt_sb[0:cqk, 512 * b:512 * (b + 1)],
                        start=True, stop=True, tile_position=(0, 0))
                    nc.tensor.matmul(
                        s_ps[:, 512:1024],
                        kf_sb[cqk:2 * cqk, P * m1_:P * (m1_ + 1)],
                        qt_sb[cqk:2 * cqk, 512 * b:512 * (b + 1)],
                        start=True, stop=True, tile_position=(cqk, 0))
                    nc.scalar.activation(flat[:, 1024 * t:1024 * (t + 1)],
                                         s_ps[:], EXP)

                def tree_q(b, i):
                    """Quarter y-sum: runs as soon as pairs 4i..4i+3 of block
                    b have exp'd, overlapping the rest of the score phase."""
                    if i == 0:
                        scr[b] = scrp.tile([P, xt // 2, 512], BF16, tag="scr",
                                           name=f"scr{b}")
                    s = scr[b]
                    nc.vector.tensor_tensor(
                        s[:, 4 * i:4 * i + 4], ptb[b][:, 8 * i:8 * i + 4],
                        ptb[b][:, 8 * i + 4:8 * i + 8], ADD)

                def tree_folds(b):
                    """Fold the four quarter-sums down to z1 = scr[b][:, 0]."""
                    s = scr[b]
                    nc.vector.tensor_tensor(s[:, 0:4], s[:, 0:4], s[:, 4:8],
                                            ADD)
                    nc.vector.tensor_tensor(s[:, 8:12], s[:, 8:12],
                                            s[:, 12:16], ADD)
                    nc.vector.tensor_tensor(s[:, 0:4], s[:, 0:4], s[:, 8:12],
                                            ADD)
                    nc.vector.tensor_tensor(s[:, 0:2], s[:, 0:2], s[:, 2:4],
                                            ADD)
                    nc.vector.tensor_tensor(s[:, 0:1], s[:, 0:1], s[:, 1:2],
                                            ADD)

                def zc_recip(b):
                    """Combine partition y-halves on PE, then reciprocal.
                    Z output borrows a psS slot (psA is full of M1 banks)."""
                    z_ps = psS.tile([P, 1024], F32, tag="sps", name=f"zps{b}")
                    nc.tensor.matmul(z_ps[:, 0:512], comb_sb[:], scr[b][:, 0],
                                     start=True, stop=True)
                    r = rzp.tile([P, 1, 512], BF16, tag="rz", name=f"rz{b}")
                    rz[b] = r
                    nc.vector.reciprocal(r[:], z_ps[:, 0:512])

                def norm_one(b):
                    """P~ *= 1/Z. One instruction: readers of ptb wait for
                    every write to it anyway (whole-tile dependencies), so
                    chunking buys nothing - instead this is emitted early
                    enough to finish during the attend's ch-major segment."""
                    nc.vector.tensor_tensor(
                        ptb[b][:], ptb[b][:],
                        rz[b][:].to_broadcast([P, xt, 512]), MULT)

                def att2_store(b, m1_sb):
                    rg = rgp.tile([P, ck, 512], BF16, tag="rg", name=f"rg{b}")
                    ld.dma_start(rg[:], rgb_t[:, :, 512 * b:512 * (b + 1)])
                    o_sb = op.tile([P, ck, 512], F32, tag="o", name=f"o{b}")
                    for ct in range(ck):
                        a_ps = psA.tile([P, 512], F32, tag="aps",
                                        name=f"aps{b}_{ct}")
                        for ch in range(ck):
                            nc.tensor.matmul(
                                a_ps[:], wv_sb[:, ch, P * ct:P * (ct + 1)],
                                m1_sb[:, ch],
                                start=(ch == 0), stop=(ch == ck - 1))
                        nc.vector.tensor_tensor(o_sb[:, ct], a_ps[:],
                                                rg[:, ct], ADD)
                        if ct % 2 == 1:  # store in halves to start earlier
                            nc.sync.dma_start(
                                out_t[:, ct - 1:ct + 1,
                                      512 * b:512 * (b + 1)],
                                o_sb[:, ct - 1:ct + 1])

                # prologue: block 0 softmax + first half of block 1 scores
                for t in range(16):
                    score_pair(0, t)
                    if t % 4 == 3:
                        tree_q(0, t // 4)
                tree_folds(0)
                for t in range(4):
                    score_pair(1, t)
                tree_q(1, 0)
                zc_recip(0)
                for t in range(4, 8):
                    score_pair(1, t)
                tree_q(1, 1)
                norm_one(0)

                # steady loop (lookahead-1): weave next block's score pairs
                # into this block's attend matmuls; M1 is m-major for the
                # first 16 m (racing the norm chunks) then ch-major so the
                # PSUM->SBUF copies stagger.
                weave = {b: [(b + 1, t) for t in range(16)] for b in range(nb)}
                weave[0] = [(1, t) for t in range(8, 16)]
                weave[nb - 1] = []
                for b in range(nb):
                    m1_sb = m1p.tile([P, ck, 512], BF16, tag="m1",
                                     name=f"m1_{b}")
                    pv = weave[b]
                    pi = 0
                    m_ps = [psA.tile([P, 512], F32, tag="aps",
                                     name=f"mps{b}_{ch}") for ch in range(ck)]
                    for m in range(16):
                        if pi < len(pv):
                            nb_, t = pv[pi]
                            score_pair(nb_, t)
                            if t % 4 == 3:
                                tree_q(nb_, t // 4)
                            pi += 1
                        for ch in range(ck):
                            nc.tensor.matmul(
                                m_ps[ch][:], chmT[:, m, ch, :], ptb[b][:, m],
                                start=(m == 0), stop=False)
                    if b + 1 < nb:
                        tree_folds(b + 1)
                    for ch in range(ck):
                        for m in range(16, xt):
                            nc.tensor.matmul(
                                m_ps[ch][:], chmT[:, m, ch, :], ptb[b][:, m],
                                start=False, stop=(m == xt - 1))
                            # hoist the Z pipeline of the next block into the
                            # middle of this block's attend so the normalize
                            # completes well before the next M1 starts
                            if ch == 1 and m == 20 and b + 1 < nb:
                                zc_recip(b + 1)
                                norm_one(b + 1)
                        nc.scalar.copy(m1_sb[:, ch], m_ps[ch][:])
                    att2_store(b, m1_sb)

    nc.compile()
    return nc


_NC_CACHE = {}


def _get_nc():
    if "nc" not in _NC_CACHE:
        _NC_CACHE["nc"] = build_program()
    return _NC_CACHE["nc"]


def _bf16(a):
    import ml_dtypes
    return np.ascontiguousarray(a.astype(ml_dtypes.bfloat16))


def make_in_maps(rgb_features, chm_features, Wq, bq, Wk, bk, Wv, bv, gamma):
    rgb_features = np.asarray(rgb_features, dtype=np.float32)
    chm_features = np.asarray(chm_features, dtype=np.float32)
    Wq = np.asarray(Wq, dtype=np.float32)
    Wk = np.asarray(Wk, dtype=np.float32)
    Wv = np.asarray(Wv, dtype=np.float32)
    bq = np.asarray(bq, dtype=np.float32)
    bk = np.asarray(bk, dtype=np.float32)
    bv = np.asarray(bv, dtype=np.float32)
    g = float(np.asarray(gamma).reshape(-1)[0])

    wq2 = _bf16(np.concatenate([Wq.T, Wq.T], axis=1))
    wk2 = _bf16(np.concatenate([Wk.T, Wk.T], axis=1))
    wv2 = _bf16((g * Wv).T)
    # softmax rows sum to 1 per (hw, x); summing over the 64 x's makes the
    # bias term contribute exactly 64*gamma*bv[c] to every output pixel.
    rgb_adj = rgb_features + (64.0 * g * bv)[None, :, None, None]
    bq2 = np.ascontiguousarray(np.concatenate([bq, bq]).reshape(2 * CQK, 1))
    bk2 = np.ascontiguousarray(np.concatenate([bk, bk]).reshape(2 * CQK, 1))
    # comb[p, i] = (p % 64 == i % 64): folds the two partition y-halves of
    # the tree result and replicates across both halves in one matmul.
    comb = _bf16(np.tile(np.eye(CQK, dtype=np.float32), (2, 2)))

    in_maps = []
    for core in range(N_CORES):
        b, half = divmod(core, 2)
        rgb_c = _bf16(
            rgb_adj[b].reshape(C, HW)[:, half * HWC:(half + 1) * HWC])
        # chm with xy permuted to (y, x) order: col' = y*64 + x.
        chm_c = _bf16(chm_features[b].reshape(C, H, W)
                      .transpose(0, 2, 1).reshape(C, XY))
        in_maps.append({
            "rgb": rgb_c, "chm": chm_c,
            "wq": wq2, "wk": wk2, "wv": wv2,
            "bq": bq2, "bk": bk2, "comb": comb,
        })
    return in_maps


def assemble(results):
    fused = np.empty((B, C, H, W), dtype=np.float32)
    fused2 = fused.reshape(B, C, HW)
    for core in range(N_CORES):
        b, half = divmod(core, 2)
        fused2[b, :, half * HWC:(half + 1) * HWC] = results[core]["out"]
    return fused


def kernel(rgb_features, chm_features, Wq, bq, Wk, bk, Wv, bv, gamma):
    nc = _get_nc()
    in_maps = make_in_maps(rgb_features, chm_features, Wq, bq, Wk, bk, Wv, bv,
                           gamma)
    res = run_bass_kernel_spmd(nc, in_maps, core_ids=list(range(N_CORES)))
    return assemble(res.results)
